# revision 118
# baseline (speedup 1.0000x reference)
"""Trainium2 Bass kernel: 16-head MHA forward (B=2, S=2048, D=1024, HD=64).

Sharding: 8 cores, each core owns 2 heads x both batches (head-parallel).
Per core: QKV projection for its heads, fused transposed-score attention
fully on-chip, output projection against its 128 rows of Wo. Host sums the
8 partial outputs and adds bo.

v3 datapath:
  - QKV projection in compensated fp8 (hi+lo e4m3 split of x and 32x-scaled
    Wqkv, 3 DoubleRow matmuls per chunk-pair = 0.75x the bf16 PE cost with
    better-than-bf16 accuracy); the 2^-5 descale folds into the bias add.
  - scores^T per (b, h, kj-tile, q-chunk) block: bf16 PE matmul,
    contraction HD=64, psum [128, 1024] = 2 blocks -> one exp per tile
  - exp on ACT is the critical path (~133us); the Exp table is preloaded
    during the initial DMA wait
  - AV accumulates BOTH heads into one [128, 130] psum (h0 cols 0..64,
    h1 cols 65..129, ones columns at 64/129 give the softmax denominators);
    one DVE reciprocal [128,2] + one broadcast tensor_tensor normalize per
    (b, qt) writes vals [128, 128] (h0|h1 features)
  - exp emission is (b, q-block)-group-major with heads interleaved per
    k-chunk, so each group's AV can run during the following group and the
    final group's AV is pre-accumulated over k-chunks 0-2, leaving only the
    last k-chunk's 8 small matmuls after the final exp
  - output projection: stationary valsT (PE transpose), moving Wo rows;
    both 512-col halves drain into one [128, 1024] sbuf tile and one DMA

Emission is slot-scheduled (slot = one exp tile): filler work (proj passes,
V transposes, AV ranges, out-proj) is paced between exp slots by a credit
pump so the PE never starves the ACT engine.

Self-contained: hardcodes shapes; only needs numpy + the concourse stack.
"""

import numpy as np

B, S, D, H, HD = 2, 2048, 1024, 16, 64
NCORES = 8
HPC = H // NCORES          # heads per core = 2
FPC = HPC * 3 * HD         # Wqkv rows per core = 384
VPC = HPC * HD             # value features per core = 128
KD = D // 128              # d-chunks = 8
ST = S // 128              # s-tiles of 128 = 16
SC = S // 512              # s-chunks of 512 = 4

_BUILT = {}


def _build(reps=1):
    if reps in _BUILT:
        return _BUILT[reps]

    import concourse.tile as tile
    import concourse.mybir as mybir
    from concourse import bacc
    from concourse.masks import make_identity

    F32 = mybir.dt.float32
    BF16 = mybir.dt.bfloat16
    FP8 = mybir.dt.float8e4
    EXP = mybir.ActivationFunctionType.Exp
    DR = mybir.MatmulPerfMode.DoubleRow

    nc = bacc.Bacc("TRN2", target_bir_lowering=False, debug=False, num_devices=1)

    xh = nc.dram_tensor("xh", [B, D, S], FP8, kind="ExternalInput").ap()
    xl = nc.dram_tensor("xl", [B, D, S], FP8, kind="ExternalInput").ap()
    # per d-row: [g0hi(128) g0lo | g1hi g1lo | g2hi g2lo] -- hi/lo adjacent
    # so one DMA per feature group moves 256B lines
    wq2 = nc.dram_tensor("wq2", [D, 3, 256], FP8, kind="ExternalInput").ap()
    bq = nc.dram_tensor("bq", [128, 3], F32, kind="ExternalInput").ap()
    woT = nc.dram_tensor("woT", [VPC, D], BF16, kind="ExternalInput").ap()
    outp = nc.dram_tensor("outp", [B, S, D], BF16,
                          kind="ExternalOutput").ap()

    with tile.TileContext(nc) as tc:
        with (
            tc.tile_pool(name="const", bufs=1) as cpool,
            tc.tile_pool(name="sb", bufs=1) as sb,
            tc.tile_pool(name="ps", bufs=1, space="PSUM") as ps,
        ):
            ident = cpool.tile([128, 128], BF16, name="ident")
            make_identity(nc, ident)
            ones2 = nc.const_aps.tensor(1.0, (128, ST, 2), BF16)

            # preload the Exp activation table during the DMA wait; the
            # LoadActFuncSet this triggers costs ~1.3us that would otherwise
            # land on the first scores exp
            dumm = cpool.tile([128, 1], F32, name="dumm")
            nc.vector.memset(dumm, 0.0)
            dummo = cpool.tile([128, 1], BF16, name="dummo")
            nc.scalar.activation(dummo, dumm, EXP)

            # descale const for the fp8 projection (weights pre-scaled 32x)
            dsc = cpool.tile([128, 1], F32, name="dsc")
            nc.vector.memset(dsc, 1.0 / 32.0)

            # PE warm-up during the initial DMA wait: keeps the tensor
            # engine continuously busy (and its p-state ramp alive) until
            # the first x00 quarter lands, so the first real matmuls run at
            # full clock
            warm_in = cpool.tile([128, 512], BF16, name="warm_in")
            nc.vector.memset(warm_in, 0.0)
            for _w in range(15):
                warm_ps = ps.tile([128, 512], mybir.dt.float32, tag="proj",
                                  bufs=2, name=f"warm_ps{_w}")
                nc.tensor.matmul(warm_ps, warm_in[:, 0:128],
                                 warm_in, start=True, stop=True)

            bq_sb = cpool.tile([128, 3], F32, name="bq_sb")
            nc.gpsimd.dma_start(out=bq_sb, in_=bq)
            # [128, k, g, hi(128)|lo(128)]
            wq_sb = cpool.tile([128, KD, 3, 256], FP8, name="wq_sb")
            wq_src = wq2.rearrange("(k p) g f -> p k g f", p=128)
            wo_sb = cpool.tile([VPC, D], BF16, name="wo_sb")

            for _rep in range(reps):
                qkv = {}     # (b, g, sc) -> [128, 512] bf16; g=0 q, 1 k, 2 v
                             # partitions: [h0 | h1] x 64 dims, type-major
                vaug = {}    # b -> [128 kj, ST, 130] bf16; h0 0:64, ones 64,
                             # h1 65:129, ones 129
                atile = {}   # tile idx -> [128, 1024] bf16 exp outputs
                vals = {}    # (b, qt) -> [128, 128] bf16 (h0|h1 features)
                valsT = {}   # (b, qt) -> [128 f, 128 q] bf16
                vo = {}      # (b, qt) -> [128, 130] f32 psum accum
                proj_state = {}

                r = f"r{_rep}"

                # ---- exp block stream -------------------------------------
                # per (b, qb) group: for kc 0..3: for h 0,1: kj 4kc..4kc+3
                # -> 32 blocks; groups ordered b0 qb0-3 then b1 qb0-3.
                blocks = []
                for b in range(B):
                    for qb in range(SC):
                        for kc in range(4):
                            for h in range(HPC):
                                for kj in range(4 * kc, 4 * kc + 4):
                                    blocks.append((b, h, qb, kj))
                NBLK = len(blocks)            # 256
                BPT = 2                       # blocks per exp tile
                NTILES = NBLK // BPT          # 128
                bidx = {blk: i for i, blk in enumerate(blocks)}

                def blk_ap(b, h, qb, kj, ql=None):
                    """aT stationary slice for an exp block (optionally one
                    128-q sub-column)."""
                    i = bidx[(b, h, qb, kj)]
                    a = atile[i // BPT]
                    c = (i % BPT) * 512
                    if ql is None:
                        return a[:, c:c + 512]
                    return a[:, c + ql * 128:c + (ql + 1) * 128]

                def x_dma(b, sc):
                    # both halves ride the SP hwdge queue: its serialized
                    # descriptor-gen paces transfers in emission order, so
                    # these never jump ahead of earlier critical DMAs
                    x_t = sb.tile([128, KD, 2, 512], FP8, tag="xt", bufs=6,
                                  name=f"xt{r}_{b}_{sc}")
                    ss = slice(sc * 512, (sc + 1) * 512)
                    xrh = xh[b].rearrange("(k p) s -> p k s", p=128)
                    xrl = xl[b].rearrange("(k p) s -> p k s", p=128)
                    nc.sync.dma_start(out=x_t[:, :, 0, :], in_=xrh[:, :, ss])
                    nc.sync.dma_start(out=x_t[:, :, 1, :], in_=xrl[:, :, ss])
                    proj_state[(b, sc)] = x_t

                # one projection pass: feature group g (0=q, 1=k, 2=v) for
                # one 512-token chunk; compensated-fp8 DoubleRow matmuls
                def proj_pass(b, g, sc, kp0, kp1):
                    x_t = proj_state[(b, sc)]
                    key = (b, g, sc)
                    if kp0 == 0:
                        proj_state[key] = ps.tile(
                            [128, 512], mybir.dt.float32, tag="proj", bufs=2,
                            name=f"pp{r}_{b}_{g}_{sc}")
                    pps = proj_state[key]
                    for kp in range(kp0, kp1):
                        ks = slice(2 * kp, 2 * kp + 2)
                        first = kp == 0
                        last = kp == KD // 2 - 1
                        wh = wq_sb[:, ks, g, 0:128]
                        wl = wq_sb[:, ks, g, 128:256]
                        nc.tensor.matmul(pps, wh, x_t[:, ks, 0, :],
                                         start=first, stop=False,
                                         perf_mode=DR)
                        nc.tensor.matmul(pps, wh, x_t[:, ks, 1, :],
                                         start=False, stop=False,
                                         perf_mode=DR)
                        nc.tensor.matmul(pps, wl, x_t[:, ks, 0, :],
                                         start=False, stop=last,
                                         perf_mode=DR)
                    if kp1 == KD // 2:
                        qkv[key] = sb.tile(
                            [128, 512], BF16, tag=f"qkv{g}", bufs=2 * SC,
                            name=f"qkv{r}_{b}_{g}_{sc}")
                        nc.vector.tensor_scalar(
                            qkv[key], pps, dsc, bq_sb[:, g:g + 1],
                            op0=mybir.AluOpType.mult,
                            op1=mybir.AluOpType.add)

                def vt_init(b):
                    va = sb.tile([128, ST, 130], BF16, tag="vaug", bufs=2,
                                 name=f"vaug{r}_{b}")
                    vaug[b] = va
                    ap = va[:, :, 64:130:65]
                    nc.vector.tensor_copy(ap, ones2)

                def vtrans(b, st0, st1):
                    va = vaug[b]
                    for st in range(st0, st1):
                        for h in range(HPC):
                            pt = ps.tile([128, HD], BF16, tag="sm",
                                         bufs=2, name=f"pt{r}_{b}_{h}_{st}")
                            vsrc = qkv[(b, 2, st // 4)][
                                h * HD:(h + 1) * HD,
                                (st % 4) * 128:(st % 4 + 1) * 128]
                            nc.tensor.transpose(
                                pt, vsrc,
                                ident[h * HD:(h + 1) * HD,
                                      h * HD:(h + 1) * HD])
                            nc.vector.tensor_copy(
                                va[:, st, 65 * h:65 * h + HD], pt)

                def scores_tile(ti):
                    s_ps = ps.tile([128, 1024], mybir.dt.float32, tag="mm",
                                   bufs=2, name=f"sps{r}_{ti}")
                    for i in range(BPT):
                        b, h, qb, kj = blocks[ti * BPT + i]
                        kT = qkv[(b, 1, kj // 4)][
                            h * HD:(h + 1) * HD,
                            (kj % 4) * 128:(kj % 4 + 1) * 128]
                        qs = qkv[(b, 0, qb)][h * HD:(h + 1) * HD, :]
                        nc.tensor.matmul(s_ps[:, i * 512:(i + 1) * 512],
                                         kT, qs, start=True, stop=True)
                    a = sb.tile([128, 1024], BF16, tag="aT", bufs=36,
                                name=f"aT{r}_{ti}")
                    atile[ti] = a
                    nc.scalar.activation(a, s_ps, EXP, scale=0.125)

                spl = {}     # (b, qt) -> [128, 130] f32 sbuf spilled partial

                def av_range(b, qt, kc0, kc1, fresh=False, close=None,
                             tag="sm"):
                    qb, ql = qt // 4, qt % 4
                    va = vaug[b]
                    if kc0 == 0 or fresh:
                        vo[(b, qt)] = ps.tile(
                            [128, 130], mybir.dt.float32, tag=tag, bufs=2,
                            name=f"vo{r}_{b}_{qt}_{kc0}")
                    v_out = vo[(b, qt)]
                    if close is None:
                        close = kc1 == 4
                    # ONE psum accumulation group per bank: start only on the
                    # very first matmul into the tile (h0 of the first kj) --
                    # h1's first write lands on pending-zero bytes -- and
                    # stop on the very last (h1 of the last kj)
                    lo = 4 * kc0 if fresh else 0
                    hi = 4 * kc1 - 1 if close else ST
                    for kj in range(4 * kc0, 4 * kc1):
                        for h in range(HPC):
                            nc.tensor.matmul(
                                v_out[:, 65 * h:65 * h + 65],
                                blk_ap(b, h, qb, kj, ql),
                                va[:, kj, 65 * h:65 * h + 65],
                                start=(kj == lo and h == 0),
                                stop=(kj == hi and h == HPC - 1))

                def av_spill(b, qt, eng="dve"):
                    # park the partial AV sum in sbuf to free the psum slot
                    # across the final exp tiles
                    spl[(b, qt)] = sb.tile([128, 130], F32, tag="spl",
                                           bufs=4, name=f"spl{r}_{b}_{qt}")
                    if eng == "act":
                        nc.scalar.activation(
                            spl[(b, qt)], vo[(b, qt)],
                            mybir.ActivationFunctionType.Copy)
                    else:
                        nc.vector.tensor_copy(spl[(b, qt)], vo[(b, qt)])

                def av_fin(b, qt):
                    v_out = vo[(b, qt)]
                    if (b, qt) in spl:
                        tot = sb.tile([128, 130], F32, tag="tot", bufs=4,
                                      name=f"tot{r}_{b}_{qt}")
                        nc.vector.tensor_tensor(tot, v_out, spl[(b, qt)],
                                                op=mybir.AluOpType.add)
                        v_out = tot
                    inv = sb.tile([128, 2], F32, tag="inv", bufs=4,
                                  name=f"inv{r}_{b}_{qt}")
                    nc.vector.reciprocal(inv, v_out[:, 64:130:65])
                    vals[(b, qt)] = sb.tile(
                        [128, 2, HD], BF16, tag="vals", bufs=8,
                        name=f"vals{r}_{b}_{qt}")
                    nc.vector.tensor_tensor(
                        vals[(b, qt)], v_out[:, 0:130:1].rearrange(
                            "p (h d) -> p h d", h=2)[:, :, 0:HD],
                        inv[:, :, None].broadcast_to((128, 2, HD)),
                        op=mybir.AluOpType.mult)

                def vals_trans(b, qt):
                    valsT[(b, qt)] = sb.tile([128, 128], BF16, tag="valsT",
                                             bufs=8,
                                             name=f"valsT{r}_{b}_{qt}")
                    tp = ps.tile([128, 128], BF16, tag="sm",
                                 bufs=2, name=f"tp{r}_{b}_{qt}")
                    vsrc = vals[(b, qt)].rearrange("p h d -> p (h d)")
                    nc.tensor.transpose(tp, vsrc, ident)
                    nc.vector.tensor_copy(valsT[(b, qt)], tp)

                cms = ("act", "dve")
                ncm = [0]

                osb = {}

                def outproj_jc(b, st, jc, tail=False):
                    if jc == 0:
                        osb[(b, st)] = sb.tile([128, 1024], BF16, tag="osb",
                                               bufs=3, name=f"osb{r}_{b}_{st}")
                    o_sb = osb[(b, st)]
                    o_ps = ps.tile([128, 512], mybir.dt.float32,
                                   tag="proj", bufs=2,
                                   name=f"ops{r}_{b}_{st}_{jc}")
                    nc.tensor.matmul(
                        o_ps, valsT[(b, st)],
                        wo_sb[:, jc * 512:(jc + 1) * 512],
                        start=True, stop=True)
                    if tail:
                        cm = cms[ncm[0] % 2]
                        ncm[0] += 1
                    else:
                        cm = "dve"
                    if cm == "act":
                        nc.scalar.activation(
                            o_sb[:, jc * 512:(jc + 1) * 512], o_ps,
                            mybir.ActivationFunctionType.Copy)
                    else:
                        nc.vector.tensor_copy(
                            o_sb[:, jc * 512:(jc + 1) * 512], o_ps)
                    if jc == 1:
                        nc.sync.dma_start(
                            out=outp[b, st * 128:(st + 1) * 128, :],
                            in_=o_sb)

                def wo_dma():
                    nc.gpsimd.dma_start(out=wo_sb, in_=woT)

                # ---------------- emission schedule ----------------
                # slot s = exp tile s; group g = slots 16g..16g+15
                # filler queue: (id, deps, min_slot, cost, closure)
                Q = []

                def fq(fid, deps, min_slot, cost, fn):
                    Q.append([fid, deps, min_slot, cost, fn])

                def gslot(b, qb, frac):
                    # slot within group (b, qb) at fraction frac
                    return (b * SC + qb) * 16 + int(frac * 16)

                fq("wo", (), 6, 0, wo_dma)

                def wqv_dma():
                    if _rep == 0:
                        nc.sync.dma_start(out=wq_sb[:, :, 2, :],
                                          in_=wq_src[:, :, 2, :])

                fq("wqv", (), 1, 0, wqv_dma)
                # b0 x chunks 1-3 staged early (needed for k passes during
                # group 0); b1 x chunks during b0 groups
                for sc in range(1, SC):
                    fq(f"xd0{sc}", (), 2 * sc - 2, 50,
                       lambda sc=sc: x_dma(0, sc))
                for sc in range(SC):
                    fq(f"xd1{sc}", (), 26 + 4 * sc, 50,
                       lambda sc=sc: x_dma(1, sc))

                # proj passes, kp-granular (4 chained units per pass) so the
                # pump can interleave them smoothly with the exp stream
                def fq_pass(b, g, sc, ms, step=1):
                    deps = (f"xd{b}{sc}",) if (sc or b) else ()
                    if g == 2:
                        deps += ("wqv",)
                    for kp in range(KD // 2):
                        fq(f"pp{b}{g}{sc}k{kp}",
                           deps if kp == 0 else (f"pp{b}{g}{sc}k{kp - 1}",),
                           ms + kp * step // 2, 340,
                           lambda b=b, g=g, sc=sc, kp=kp:
                               proj_pass(b, g, sc, kp, kp + 1))
                    fq(f"pp{b}{g}{sc}", (f"pp{b}{g}{sc}k{KD // 2 - 1}",),
                       0, 0, lambda: None)

                # b0: k1-k3 early (scores blocks need k chunks at slots
                # ~4/8/12), q1-3 before their groups, v0-3 before vtrans
                for g, sc, ms in ((1, 1, 2), (1, 2, 5), (1, 3, 9),
                                  (0, 1, 9), (2, 0, 10), (2, 1, 11),
                                  (2, 2, 12), (2, 3, 13), (0, 2, 16),
                                  (0, 3, 30)):
                    fq_pass(0, g, sc, ms)
                # b1 proj passes: v first (they feed the vt1 window at
                # 44-49, before group 2's AV chains own the sm pool), then
                # k + q0 before group 4 (slot 64)
                for g, sc, ms in ((2, 0, 36), (2, 1, 40), (2, 2, 44),
                                  (2, 3, 46), (1, 0, 48), (1, 1, 50),
                                  (1, 2, 52), (1, 3, 54), (0, 0, 52),
                                  (0, 1, 66), (0, 2, 82), (0, 3, 98)):
                    fq_pass(1, g, sc, ms)

                # V transposes: 2-tile chunks, after the v chunk projects.
                # Their psum tiles share the "sm" pool with the AV
                # accumulators, so they are scheduled in windows where no AV
                # chain holds a slot (b0: before the first chains; b1: the
                # chain-free zone before group 3's chains start)
                vt_slots = {0: (11, 11, 12, 12, 13, 13, 14, 14),
                            1: (44, 44, 45, 45, 46, 47, 48, 49)}
                for b in range(B):
                    fq(f"vi{b}", (), vt_slots[b][0] - 1, 50,
                       lambda b=b: vt_init(b))
                    for c8 in range(8):
                        fq(f"vt{b}{c8}", (f"vi{b}", f"pp{b}2{c8 // 2}"),
                           vt_slots[b][c8], 150,
                           lambda b=b, c8=c8: vtrans(b, 2 * c8, 2 * c8 + 2))
                    fq(f"vt{b}", tuple(f"vt{b}{c8}" for c8 in range(8)),
                       0, 0, lambda: None)

                # AV + chains: group (b, qb)'s AV runs during the next
                # group, split in two ranges so the pump can pace it; the
                # final group pre-accumulates kc0-2, spills to sbuf, and
                # finishes kc3 after the last exp (split across both psum
                # pools so four accumulators can coexist)
                for b in range(B):
                    for qt in range(ST):
                        qb = qt // 4
                        g = b * SC + qb
                        tail = g == 2 * SC - 1
                        vdep = (f"vt{b}",)
                        if g == 2 * SC - 2:
                            # penultimate group: accumulate kc0-1 within the
                            # group (exps land at 16g+8), spill, and finish
                            # kc2-3 fresh once they land at 16g+16 -- so all
                            # its chains clear before the tail pre-AV zone
                            s0 = 16 * g + 8 + 2 * (qt % 4)
                            fq(f"av{b}{qt}a", vdep, s0, 640,
                               lambda b=b, qt=qt:
                                   (av_range(b, qt, 0, 2, close=True),
                                    av_spill(b, qt)))
                            fq(f"av{b}{qt}", (f"av{b}{qt}a",),
                               16 * g + 16 + 2 * (qt % 4), 440,
                               lambda b=b, qt=qt:
                                   av_range(b, qt, 2, 4, fresh=True))
                        elif not tail:
                            s0 = 16 * g + 17 + 3 * (qt % 4)
                            fq(f"av{b}{qt}a", vdep, s0, 440,
                               lambda b=b, qt=qt: av_range(b, qt, 0, 2))
                            fq(f"av{b}{qt}", (f"av{b}{qt}a",), s0 + 1, 440,
                               lambda b=b, qt=qt: av_range(b, qt, 2, 4))
                        else:
                            s0 = 16 * g + 8 + 2 * (qt % 4)
                            pt = "sm" if qt % 4 < 2 else "proj"
                            fq(f"av{b}{qt}a", vdep, s0, 440,
                               lambda b=b, qt=qt, pt=pt:
                                   av_range(b, qt, 0, 2, tag=pt))
                            fq(f"av{b}{qt}p", (f"av{b}{qt}a",),
                               16 * g + 12 + (qt % 4), 280,
                               lambda b=b, qt=qt:
                                   (av_range(b, qt, 2, 3, close=True),
                                    av_spill(b, qt)))
                            fq(f"av{b}{qt}", (f"av{b}{qt}p",), NTILES, 120,
                               lambda b=b, qt=qt:
                                   av_range(b, qt, 3, 4, fresh=True))
                        # tail chains sort AFTER the fresh-AV units so the
                        # drain interleaves all four PE ranges up front
                        # while the DVE chains pipeline behind them
                        ms2 = NTILES + 1 if tail else s0 + 2
                        fq(f"fn{b}{qt}", (f"av{b}{qt}",), ms2, 60,
                           lambda b=b, qt=qt: av_fin(b, qt))
                        fq(f"tr{b}{qt}", (f"fn{b}{qt}",), ms2, 100,
                           lambda b=b, qt=qt: vals_trans(b, qt))
                        for jc in range(2):
                            dep = (f"tr{b}{qt}", "wo") if jc == 0 \
                                else (f"op{b}{qt}0",)
                            fq(f"op{b}{qt}{jc}", dep, ms2 if tail else ms2 + 2, 260,
                               lambda b=b, qt=qt, jc=jc, tail=tail:
                                   outproj_jc(b, qt, jc, tail))

                Q.sort(key=lambda it: it[2])
                emitted = set()
                credit = [0.0]

                def pump(slot, budget):
                    credit[0] = min(credit[0] + budget, 2.4 * budget)
                    while credit[0] > 0:
                        pick = None
                        for item in Q:
                            fid, deps, ms, cost, fn = item
                            if ms <= slot and all(d in emitted for d in deps):
                                pick = item
                                break
                        if pick is None:
                            return
                        Q.remove(pick)
                        emitted.add(pick[0])
                        pick[4]()
                        credit[0] -= pick[3]

                def force_emit(fid):
                    if fid in emitted:
                        return
                    item = next(it for it in Q if it[0] == fid)
                    for d in item[1]:
                        force_emit(d)
                    Q.remove(item)
                    emitted.add(fid)
                    item[4]()

                # head: the first exps need k0 (wq K-cols + all of x00) and
                # q0.  DMA order is chosen so proj half-passes start as soon
                # as each x00 quarter lands; wq V-cols stream later via the
                # pump.  hi pieces ride the SP hwdge queue, lo pieces the
                # ACT queue.
                x00 = sb.tile([128, KD, 2, 512], FP8, tag="xt", bufs=6,
                              name=f"xt{r}_0_0")
                xrh0 = xh[0].rearrange("(k p) s -> p k s", p=128)
                xrl0 = xl[0].rearrange("(k p) s -> p k s", p=128)

                def wq_grp(g):
                    if _rep == 0:
                        nc.sync.dma_start(out=wq_sb[:, :, g, :],
                                          in_=wq_src[:, :, g, :])

                def x00_q(kq):
                    ks = slice(2 * kq, 2 * kq + 2)
                    nc.sync.dma_start(out=x00[:, ks, 0, :],
                                      in_=xrh0[:, ks, 0:512])
                    nc.scalar.dma_start(out=x00[:, ks, 1, :],
                                        in_=xrl0[:, ks, 0:512])

                wq_grp(1)                 # K columns
                x00_q(0)
                wq_grp(0)                 # Q columns
                x00_q(1)
                x00_q(2)
                x00_q(3)
                proj_state[(0, 0)] = x00
                # k0/q0 interleaved at pair granularity
                for kp in range(KD // 2):
                    proj_pass(0, 1, 0, kp, kp + 1)
                    proj_pass(0, 0, 0, kp, kp + 1)

                for ti in range(NTILES):
                    # make sure the q/k proj tiles this exp tile reads exist
                    for i in range(BPT):
                        b, h, qb, kj = blocks[ti * BPT + i]
                        if (b, 0, qb) not in qkv:
                            force_emit(f"pp{b}0{qb}")
                        if (b, 1, kj // 4) not in qkv:
                            force_emit(f"pp{b}1{kj // 4}")
                    scores_tile(ti)
                    pump(ti, 620)
                # tail: drain the queue in dependency order
                guard = 0
                while Q:
                    n0 = len(Q)
                    pump(10 ** 9, 10 ** 9)
                    assert len(Q) < n0 or guard < 3, \
                        f"stuck queue: {[i[0] for i in Q]}"
                    guard += 1

    nc.compile()
    _BUILT[reps] = nc
    return nc


def _in_maps(x, Wqkv, bqkv, Wo):
    import ml_dtypes
    BF = ml_dtypes.bfloat16
    F8 = ml_dtypes.float8_e4m3fn
    xT = np.ascontiguousarray(x.transpose(0, 2, 1))
    xh = xT.astype(F8)
    xl = (xT - xh.astype(np.float32)).astype(F8)
    in_maps = []
    for c in range(NCORES):
        rows = slice(c * FPC, (c + 1) * FPC)
        cols = slice(c * VPC, (c + 1) * VPC)
        # permute head-major [h0:qkv | h1:qkv] rows to type-major
        # [q_h0 q_h1 | k_h0 k_h1 | v_h0 v_h1] so q/k/v of one head share a
        # base partition on chip
        wc = Wqkv[rows].reshape(HPC, 3, HD, D).transpose(1, 0, 2, 3)
        bc = bqkv[rows].reshape(HPC, 3, HD).transpose(1, 0, 2)
        wT = np.ascontiguousarray(wc.reshape(FPC, D).T) * 32.0
        wh = wT.astype(F8)
        wl = (wT - wh.astype(np.float32)).astype(F8)
        # [D, g, hi|lo]: hi/lo adjacent per group for 256B DMA lines
        wq2 = np.concatenate(
            [np.stack([wh.reshape(D, 3, 128)[:, g], wl.reshape(D, 3, 128)[:, g]],
                      axis=1)[:, None] for g in range(3)], axis=1)
        in_maps.append({
            "xh": xh,
            "xl": xl,
            "wq2": np.ascontiguousarray(wq2.reshape(D, 3, 256)),
            "bq": np.ascontiguousarray(
                bc.reshape(3, 128).T, dtype=np.float32),
            "woT": np.ascontiguousarray(Wo[:, cols].T).astype(BF),
        })
    return in_maps


def _run_device(x, Wqkv, bqkv, Wo, trace=False):
    from concourse import bass_utils

    nc = _build()
    in_maps = _in_maps(x, Wqkv, bqkv, Wo)
    kw = {}
    if trace:
        kw = dict(trace=True, trace_cores=list(range(NCORES)),
                  stitch_traces=True)
    res = bass_utils.run_bass_kernel_spmd(
        nc, in_maps, core_ids=list(range(NCORES)), **kw)
    acc = res.results[0]["outp"].astype(np.float64)
    for c in range(1, NCORES):
        acc += res.results[c]["outp"]
    return acc, res


def _numpy_fallback(x, mask, Wqkv, bqkv, Wo, bo):
    qkv = x @ Wqkv.T + bqkv
    qkv = qkv.reshape(B, S, H, 3 * HD).transpose(0, 2, 1, 3)
    q, k, v = np.split(qkv, 3, axis=-1)
    sc = np.einsum("bhqd,bhkd->bhqk", q, k) / np.sqrt(HD).astype(np.float32)
    sc = sc + mask
    sc = sc - sc.max(axis=-1, keepdims=True)
    a = np.exp(sc)
    a /= a.sum(axis=-1, keepdims=True)
    vals = np.einsum("bhqk,bhkd->bhqd", a, v)
    vals = vals.transpose(0, 2, 1, 3).reshape(B, S, D)
    return (vals @ Wo.T + bo).astype(np.float32)


def kernel(x, mask, Wqkv, bqkv, Wo, bo):
    x = np.asarray(x, dtype=np.float32)
    mask = np.asarray(mask, dtype=np.float32)
    Wqkv = np.asarray(Wqkv, dtype=np.float32)
    bqkv = np.asarray(bqkv, dtype=np.float32)
    Wo = np.asarray(Wo, dtype=np.float32)
    bo = np.asarray(bo, dtype=np.float32)
    if mask.any():
        # device kernel folds the (all-zero) mask away; fall back if nonzero
        return _numpy_fallback(x, mask, Wqkv, bqkv, Wo, bo)
    acc, _ = _run_device(x, Wqkv, bqkv, Wo)
    return (acc + bo).astype(np.float32)
